# revision 4
# baseline (speedup 1.0000x reference)
"""KNN WRMF sampler, 8 trn2 cores — fused flat kernel (HW-valid constructs only).

Per core: 512 rows, 4 tiles of 128 partitions, row l -> (p t) layout.
One [P,1]-offset indirect gather per tile fetches a host-fused 300-float
record per row:
  [0:100]   cum row           (compares use 0:99)
  [100:199] ptab diffs        pd[n] = ptab[n+1]-ptab[n]
  [199:298] knn diffs         kd[n] = knn[n+1]-knn[n]  (exact ints in f32)
  [298:300] ptab[0], knn[0]
Then the exact flat telescoped extraction, dual-fused over both tables:
  A[n] = 1[cum[n] < u]                                  [P,K,99]
  rr   = sum_n A[n] * (pd|kd)[n]                        [P,K,2]
  out  = rr + (ptab[0]|knn[0])  == tab[min(idx,99)]     exact
Outputs stored as f32 [prob | neg] pairs; host converts neg to int32.
"""

import numpy as np
from contextlib import ExitStack

import concourse.bass as bass
import concourse.bacc as bacc
import concourse.mybir as mybir
import concourse.tile as tile
from concourse.bass_utils import run_bass_kernel_spmd

P = 128
T = 4
RPC = P * T
K = 32
N = 100
REC = 3 * N      # 300 f32 per row record
NCORES = 8
NLOC = 100000

_cache = {}


def _build():
    if "nc" in _cache:
        return _cache["nc"]
    nc = bacc.Bacc("TRN2")
    f32, i32 = mybir.dt.float32, mybir.dt.int32
    trg = nc.dram_tensor("trg", [RPC], i32, kind="ExternalInput").ap()
    uni = nc.dram_tensor("uni", [RPC, K], f32, kind="ExternalInput").ap()
    frt = nc.dram_tensor("frt", [NLOC + 1, REC], f32, kind="ExternalInput").ap()
    outp = nc.dram_tensor("outp", [RPC, K, 2], f32, kind="ExternalOutput").ap()

    GT = mybir.AluOpType.is_gt
    MUL = mybir.AluOpType.mult
    ADD = mybir.AluOpType.add
    X = mybir.AxisListType.X

    with tile.TileContext(nc) as tc, ExitStack() as ctx:
        pool = ctx.enter_context(tc.tile_pool(name="m", bufs=1))
        big = ctx.enter_context(tc.tile_pool(name="big", bufs=2))
        rp = ctx.enter_context(tc.tile_pool(name="rp", bufs=2))

        loc = pool.tile([P, T], i32)
        nc.sync.dma_start(loc[:], trg.rearrange("(p t) -> p t", p=P))
        ut = pool.tile([P, T, K], f32)
        nc.sync.dma_start(ut[:], uni.rearrange("(p t) k -> p t k", p=P))

        outv = outp.rearrange("(p t) k c -> p t (k c)", p=P)

        for t in range(T):
            rec = rp.tile([P, REC], f32, tag="rec")
            nc.gpsimd.indirect_dma_start(
                out=rec[:], out_offset=None, in_=frt[:],
                in_offset=bass.IndirectOffsetOnAxis(ap=loc[:, t:t + 1], axis=0))

            a = big.tile([P, K, N - 1], f32, tag="a")
            nc.vector.tensor_tensor(
                out=a[:],
                in0=ut[:, t, :][:, :, None].to_broadcast([P, K, N - 1]),
                in1=rec[:, None, 0:N - 1].to_broadcast([P, K, N - 1]),
                op=GT)
            dm = big.tile([P, K, 2, N - 1], f32, tag="dm")
            nc.vector.tensor_tensor(
                out=dm[:],
                in0=a[:, :, None, :].to_broadcast([P, K, 2, N - 1]),
                in1=rec[:, N:N + 2 * (N - 1)]
                    .rearrange("p (c n) -> p c n", c=2)[:, None, :, :]
                    .to_broadcast([P, K, 2, N - 1]),
                op=MUL)
            rr = big.tile([P, K, 2], f32, tag="rr")
            nc.vector.tensor_reduce(out=rr[:], in_=dm[:], axis=X, op=ADD)
            out2 = big.tile([P, K, 2], f32, tag="out2")
            nc.vector.tensor_tensor(
                out=out2[:], in0=rr[:],
                in1=rec[:, None, 3 * N - 2:3 * N].to_broadcast([P, K, 2]),
                op=ADD)
            nc.sync.dma_start(outv[:, t, :],
                              out2[:].rearrange("p k c -> p (k c)"))
    nc.compile()
    _cache["nc"] = nc
    return nc


def _host_tables(knn_results, probs_table, cum_probs_table):
    cum = np.asarray(cum_probs_table, dtype=np.float32)
    pt = np.asarray(probs_table, dtype=np.float32)
    kn = np.asarray(knn_results)

    fr = np.empty((NLOC + 1, REC), dtype=np.float32)
    fr[:, 0:N] = cum
    fr[:, N:2 * N - 1] = pt[:, 1:N] - pt[:, 0:N - 1]
    kn_f = np.zeros((NLOC + 1, N), dtype=np.float32)
    kn_f[1:] = kn.astype(np.float32)
    fr[:, 2 * N - 1:3 * N - 2] = kn_f[:, 1:N] - kn_f[:, 0:N - 1]
    fr[:, 3 * N - 2] = pt[:, 0]
    fr[:, 3 * N - 1] = kn_f[:, 0]
    return fr


def prepare_in_maps(inputs):
    loc = np.ascontiguousarray(
        np.asarray(inputs["trg_seq"])[:, 1].astype(np.int32))
    uni = np.ascontiguousarray(np.asarray(inputs["uniforms"], dtype=np.float32))
    fr = _host_tables(inputs["knn_results"], inputs["probs_table"],
                      inputs["cum_probs_table"])
    in_maps = []
    for c in range(NCORES):
        sl = slice(c * RPC, (c + 1) * RPC)
        in_maps.append({"trg": loc[sl], "uni": uni[sl], "frt": fr})
    return in_maps


def kernel(trg_seq, k, user, uniforms, knn_results, probs_table, cum_probs_table,
           **_ignored):
    nc = _build()
    in_maps = prepare_in_maps({
        "trg_seq": trg_seq, "uniforms": uniforms, "knn_results": knn_results,
        "probs_table": probs_table, "cum_probs_table": cum_probs_table,
    })
    res = run_bass_kernel_spmd(nc, in_maps, core_ids=list(range(NCORES)))
    packed = np.concatenate([res.results[c]["outp"] for c in range(NCORES)], axis=0)
    prob = np.ascontiguousarray(packed[:, :, 0])
    neg = np.rint(packed[:, :, 1]).astype(np.int32)
    return neg, prob


# revision 5
# speedup vs baseline: 1.2109x; 1.2109x over previous
"""KNN WRMF sampler, 8 trn2 cores — fused flat kernel (HW-valid constructs only).

Per core: 512 rows, 4 tiles of 128 partitions, row l -> (p t) layout.
One [P,1]-offset indirect gather per tile fetches a host-fused 300-float
record per row:
  [0:100]   cum row           (compares use 0:99)
  [100:199] ptab diffs        pd[n] = ptab[n+1]-ptab[n]
  [199:298] knn diffs         kd[n] = knn[n+1]-knn[n]  (exact ints in f32)
  [298:300] ptab[0], knn[0]
Then the exact flat telescoped extraction, dual-fused over both tables:
  A[n] = 1[cum[n] < u]                                  [P,K,99]
  rr   = sum_n A[n] * (pd|kd)[n]                        [P,K,2]
  out  = rr + (ptab[0]|knn[0])  == tab[min(idx,99)]     exact
Outputs stored as f32 [prob | neg] pairs; host converts neg to int32.
"""

import numpy as np
from contextlib import ExitStack

import concourse.bass as bass
import concourse.bacc as bacc
import concourse.mybir as mybir
import concourse.tile as tile
from concourse.bass_utils import run_bass_kernel_spmd

P = 128
T = 4
RPC = P * T
K = 32
N = 100
REC = 3 * N      # 300 f32 per row record
NCORES = 8
NLOC = 100000

_cache = {}


def _build():
    if "nc" in _cache:
        return _cache["nc"]
    nc = bacc.Bacc("TRN2")
    f32, i32 = mybir.dt.float32, mybir.dt.int32
    trg = nc.dram_tensor("trg", [RPC], i32, kind="ExternalInput").ap()
    uni = nc.dram_tensor("uni", [RPC, K], f32, kind="ExternalInput").ap()
    frt = nc.dram_tensor("frt", [NLOC + 1, REC], f32, kind="ExternalInput").ap()
    outp = nc.dram_tensor("outp", [RPC, K, 2], f32, kind="ExternalOutput").ap()

    GT = mybir.AluOpType.is_gt
    MUL = mybir.AluOpType.mult
    ADD = mybir.AluOpType.add
    X = mybir.AxisListType.X

    with tile.TileContext(nc) as tc, ExitStack() as ctx:
        pool = ctx.enter_context(tc.tile_pool(name="m", bufs=1))
        big = ctx.enter_context(tc.tile_pool(name="big", bufs=2))
        rp = ctx.enter_context(tc.tile_pool(name="rp", bufs=2))

        loc = pool.tile([P, T], i32)
        nc.sync.dma_start(loc[:], trg.rearrange("(p t) -> p t", p=P))
        ut = pool.tile([P, T, K], f32)
        nc.sync.dma_start(ut[:], uni.rearrange("(p t) k -> p t k", p=P))

        outv = outp.rearrange("(p t) k c -> p t (k c)", p=P)

        for t in range(T):
            rec = rp.tile([P, REC], f32, tag="rec")
            nc.gpsimd.indirect_dma_start(
                out=rec[:], out_offset=None, in_=frt[:],
                in_offset=bass.IndirectOffsetOnAxis(ap=loc[:, t:t + 1], axis=0))

            a = big.tile([P, K, N - 1], f32, tag="a")
            nc.vector.tensor_tensor(
                out=a[:],
                in0=ut[:, t, :][:, :, None].to_broadcast([P, K, N - 1]),
                in1=rec[:, None, 0:N - 1].to_broadcast([P, K, N - 1]),
                op=GT)
            # knn chain on DVE, prob-masking on the otherwise idle GPSIMD
            dm = big.tile([P, K, 2, N - 1], f32, tag="dm")
            nc.gpsimd.tensor_tensor(
                out=dm[:, :, 0, :],
                in0=a[:],
                in1=rec[:, None, N:2 * N - 1].to_broadcast([P, K, N - 1]),
                op=MUL)
            nc.vector.tensor_tensor(
                out=dm[:, :, 1, :],
                in0=a[:],
                in1=rec[:, None, 2 * N - 1:3 * N - 2]
                    .to_broadcast([P, K, N - 1]),
                op=MUL)
            rr = big.tile([P, K, 2], f32, tag="rr")
            nc.vector.tensor_reduce(out=rr[:], in_=dm[:], axis=X, op=ADD)
            out2 = big.tile([P, K, 2], f32, tag="out2")
            nc.vector.tensor_tensor(
                out=out2[:], in0=rr[:],
                in1=rec[:, None, 3 * N - 2:3 * N].to_broadcast([P, K, 2]),
                op=ADD)
            nc.sync.dma_start(outv[:, t, :],
                              out2[:].rearrange("p k c -> p (k c)"))
    nc.compile()
    _cache["nc"] = nc
    return nc


def _host_tables(knn_results, probs_table, cum_probs_table):
    cum = np.asarray(cum_probs_table, dtype=np.float32)
    pt = np.asarray(probs_table, dtype=np.float32)
    kn = np.asarray(knn_results)

    fr = np.empty((NLOC + 1, REC), dtype=np.float32)
    fr[:, 0:N] = cum
    fr[:, N:2 * N - 1] = pt[:, 1:N] - pt[:, 0:N - 1]
    kn_f = np.zeros((NLOC + 1, N), dtype=np.float32)
    kn_f[1:] = kn.astype(np.float32)
    fr[:, 2 * N - 1:3 * N - 2] = kn_f[:, 1:N] - kn_f[:, 0:N - 1]
    fr[:, 3 * N - 2] = pt[:, 0]
    fr[:, 3 * N - 1] = kn_f[:, 0]
    return fr


def prepare_in_maps(inputs):
    loc = np.ascontiguousarray(
        np.asarray(inputs["trg_seq"])[:, 1].astype(np.int32))
    uni = np.ascontiguousarray(np.asarray(inputs["uniforms"], dtype=np.float32))
    fr = _host_tables(inputs["knn_results"], inputs["probs_table"],
                      inputs["cum_probs_table"])
    in_maps = []
    for c in range(NCORES):
        sl = slice(c * RPC, (c + 1) * RPC)
        in_maps.append({"trg": loc[sl], "uni": uni[sl], "frt": fr})
    return in_maps


def kernel(trg_seq, k, user, uniforms, knn_results, probs_table, cum_probs_table,
           **_ignored):
    nc = _build()
    in_maps = prepare_in_maps({
        "trg_seq": trg_seq, "uniforms": uniforms, "knn_results": knn_results,
        "probs_table": probs_table, "cum_probs_table": cum_probs_table,
    })
    res = run_bass_kernel_spmd(nc, in_maps, core_ids=list(range(NCORES)))
    packed = np.concatenate([res.results[c]["outp"] for c in range(NCORES)], axis=0)
    prob = np.ascontiguousarray(packed[:, :, 0])
    neg = np.rint(packed[:, :, 1]).astype(np.int32)
    return neg, prob
